# revision 32
# baseline (speedup 1.0000x reference)
"""Trainium2 Bass kernel for multi-head attention (b=4, n=2048, d=512, h=8, dk=dv=64).

Sharding: 8 cores = 4 batches x 2 query-halves. Each core computes K/V for its
full batch sequence (2048) and attention outputs for its 1024 query rows.
No collectives needed; host stacks the per-core [1024, 512] outputs.

Per-core dataflow (f32r = TF32-like fast fp32 matmul mode; PV in bf16):
  x^T [512, 2048] staged in SBUF.
  Q^T per head, replicated to both partition halves: qt2 [128, h, i]
  K^T per head stored block-diagonally per j-chunk: kt_bd [128, h, jc, 128]
    rows 0:64  = K^T dims x even 64-j half   (cols 0:64)
    rows 64:128= K^T dims x odd 64-j half    (cols 64:128), zeros elsewhere
  -> S^T matmul has K=128 (full PE rate): out rows = 128 consecutive j's.
  V   = x Wv  (+ ones col) [per j-chunk: 128j, 8h*65] in bf16
  c   = x (Wk_h @ rel_bias_h) [per j-chunk: 128j, 8h]  (bias term of logits)
  P^T = exp(S^T + c[j])   (no max-subtraction: logits < ~50)
  outT[65, i] accumulated over j-chunks via lhsT=V_aug, rhs=P^T; row 64 = denom
  out^T scaled by 1/denom (reciprocal_approx_fast + partition broadcast)
  y = out^T.T @ Wo + bo  (bias via broadcast bo + DVE add on PSUM copy).
"""
import numpy as np

B, N, MODEL = 4, 2048, 512
H, DK = 8, 64
SCALE = DK ** -0.5
NI = 1024          # query rows per core
NCH = MODEL // 128  # model-dim chunks
NJC = N // 128      # key/value chunks
NHP = H // 2        # head pairs
JBATCH = 4          # j-chunks per ST/PV batch

_COMPILED = None


def _build():
    import concourse.bass as bass
    from concourse import bacc
    import concourse.mybir as mybir
    import concourse.tile as tile

    F32 = mybir.dt.float32
    F32R = mybir.dt.float32r
    BF16 = mybir.dt.bfloat16
    EXP = mybir.ActivationFunctionType.Exp

    nc = bacc.Bacc("TRN2", target_bir_lowering=False, debug=False, num_devices=8)
    xt_in = nc.dram_tensor("xt", [MODEL, N], F32R, kind="ExternalInput")
    wq_in = nc.dram_tensor("wq", [MODEL, H * 128], F32R, kind="ExternalInput")
    wk_in = nc.dram_tensor("wk", [MODEL, H * 128], F32R, kind="ExternalInput")
    wv_in = nc.dram_tensor("wv", [MODEL, MODEL], F32R, kind="ExternalInput")
    relb_in = nc.dram_tensor("relb", [128, H], F32, kind="ExternalInput")
    wo_in = nc.dram_tensor("wo", [MODEL, MODEL], F32R, kind="ExternalInput")
    bo_in = nc.dram_tensor("bo", [1, MODEL], F32, kind="ExternalInput")
    onesb_in = nc.dram_tensor("onesb", [128, NJC * H], BF16, kind="ExternalInput")
    y_out = nc.dram_tensor("y", [NI, MODEL], F32, kind="ExternalOutput")

    HALF = N // 2

    with tile.TileContext(nc) as tc:
        with (
            tc.tile_pool(name="w", bufs=1) as wp,
            tc.tile_pool(name="acts", bufs=1) as ap,
            tc.tile_pool(name="big", bufs=2, space="PSUM") as ps,
            tc.tile_pool(name="qk", bufs=1, space="PSUM") as qkp,
            tc.tile_pool(name="pv", bufs=1, space="PSUM") as pvp,
        ):
            # ---------- persistent tiles ----------
            wo = wp.tile([128, NCH, MODEL], F32R, tag="wo")
            bo = wp.tile([1, MODEL], F32, tag="bo")
            bo_b = wp.tile([128, MODEL], F32, tag="bo_b")
            vv = ap.tile([128, NJC, H * 65], BF16, tag="vv")
            relb = ap.tile([128, H], F32, tag="relb")
            outt = ap.tile([128, NHP, NI], F32R, tag="outt")
            xt0 = ap.tile([128, NCH, 512], F32R, tag="xt0")
            xt1 = ap.tile([128, NCH, 512], F32R, tag="xt1")
            xt2 = ap.tile([128, NCH, 512], F32R, tag="xt2")
            xt3 = ap.tile([128, NCH, 512], F32R, tag="xt3")
            xts = [xt0, xt1, xt2, xt3]
            wq = ap.tile([128, NCH, H * 128], F32R, tag="wq")
            wk = ap.tile([128, NCH, H * 128], F32R, tag="wk")

            def r3(d):
                return d[:].rearrange("(c p) n -> p c n", p=128)

            def dma(i, out, in_, engs=None):
                engs = engs or (nc.sync, nc.gpsimd)
                engs[i % len(engs)].dma_start(out=out, in_=in_)

            xsrc = r3(xt_in)
            for q in range(4):
                for chh in range(2):
                    dma(q * 2 + chh,
                        xts[q][:, chh * 2:(chh + 1) * 2, :],
                        xsrc[:, chh * 2:(chh + 1) * 2, q * 512:(q + 1) * 512],
                        engs=(nc.sync, nc.gpsimd, nc.scalar))
            nc.sync.dma_start(out=bo[:], in_=bo_in[:])
            nc.gpsimd.partition_broadcast(bo_b[:], bo[:])
            # ones columns of V_aug: contiguous DMA to scratch, strided DVE copy
            onesb_t = wp.tile([128, NJC * H], BF16, tag="onesb")
            nc.sync.dma_start(out=onesb_t[:], in_=onesb_in[:])
            nc.vector.tensor_copy(
                vv[:].rearrange("p j (h e) -> p (j h) e", e=65)[:, :, 64:65],
                onesb_t[:].rearrange("p (n o) -> p n o", o=1))
            def xtv(ch, start, size):
                t = xts[start // 512]
                off = start % 512
                assert off + size <= 512
                return t[:, ch, off:off + size]


            kt_t0 = ap.tile([128, NJC, 128], F32R, tag="kt0")
            kt_t1 = ap.tile([128, NJC, 128], F32R, tag="kt1")
            qt_t0 = ap.tile([128, NI], F32R, tag="qt0")
            qt_t1 = ap.tile([128, NI], F32R, tag="qt1")
            with (
                tc.tile_pool(name="pt", bufs=10) as ptp,
                tc.tile_pool(name="norm", bufs=2) as np_,
                tc.tile_pool(name="ysb", bufs=2) as yp_sb,
            ):
                # ---- V and c projections (vv also carries the exp bias c) ----
                with tc.tile_pool(name="s1v", bufs=1) as s1v:
                    wv = s1v.tile([128, NCH, MODEL], F32R, tag="wv")

                    zeros = s1v.tile([128, 512], F32, tag="zeros")
                    nc.vector.memset(zeros[:], 0.0)
                    for ktz in (kt_t0, kt_t1):
                        for jh in range(2):
                            js = slice(jh * 8, jh * 8 + 8)
                            nc.vector.tensor_copy(
                                ktz[0:64, js, 64:128],
                                zeros[0:64].rearrange("p (j m) -> p j m", m=64))
                            nc.vector.tensor_copy(
                                ktz[64:128, js, 0:64],
                                zeros[64:128].rearrange("p (j m) -> p j m", m=64))
                    for ch in range(NCH):
                        dma(ch, wv[:, ch], r3(wv_in)[:, ch])
                    nc.sync.dma_start(out=relb[:], in_=relb_in[:])
                    for ch in range(NCH):
                        dma(ch, wq[:, ch], r3(wq_in)[:, ch])
                        dma(ch + 1, wk[:, ch], r3(wk_in)[:, ch])
                        dma(ch, wo[:, ch], r3(wo_in)[:, ch])
                    for jc in range(NJC):
                        v_ps = ps.tile([128, NI], F32, tag="big")
                        for ch in range(NCH):
                            nc.tensor.matmul(v_ps[:, 0:MODEL],
                                             xtv(ch, jc * 128, 128),
                                             wv[:, ch],
                                             start=(ch == 0), stop=(ch == NCH - 1))
                        nc.vector.tensor_copy(
                            vv[:, jc].rearrange("p (h e) -> p h e", e=65)[:, :, 0:64],
                            v_ps[:, 0:MODEL].rearrange("p (h e) -> p h e", e=64))

                    # ---- merged per-head: Q^T, K^T(block-diag), ST/exp/PV ----
                    def emit_q(h):
                        qt = qt_t0 if h % 2 == 0 else qt_t1
                        q_ps = qkp.tile([128, NI], F32, tag="qk")
                        for ib in range(NI // 512):
                            for ch in range(NCH):
                                nc.tensor.matmul(
                                    q_ps[:, ib * 512:(ib + 1) * 512],
                                    wq[:, ch, h * 128:(h + 1) * 128],
                                    xtv(ch, ib * 512, 512),
                                    start=(ch == 0), stop=(ch == NCH - 1))
                        nc.vector.tensor_scalar_add(qt[:], q_ps[:], relb[:, h:h + 1])

                    def emit_k(h):
                        kt = kt_t0 if h % 2 == 0 else kt_t1
                        for jb in range(N // NI):
                            k_ps = qkp.tile([128, NI], F32, tag="qk")
                            for sb in range(NI // 512):
                                off = jb * NI + sb * 512
                                for ch in range(NCH):
                                    nc.tensor.matmul(
                                        k_ps[:, sb * 512:(sb + 1) * 512],
                                        wk[:, ch, h * 128:(h + 1) * 128],
                                        xtv(ch, off, 512),
                                        start=(ch == 0), stop=(ch == NCH - 1))
                            kp = k_ps[:].rearrange("p (t e c) -> p t e c", t=8, e=2)
                            jcs = slice(jb * 8, jb * 8 + 8)
                            nc.vector.tensor_copy(kt[0:64, jcs, 0:64],
                                                  kp[0:64, :, 0])
                            nc.vector.tensor_copy(kt[64:128, jcs, 64:128],
                                                  kp[64:128, :, 1])

                    emit_q(0)
                    emit_k(0)
                    for h in range(H):
                        hp, hr = h // 2, (h % 2) * 64
                        qt = qt_t0 if h % 2 == 0 else qt_t1
                        kt = kt_t0 if h % 2 == 0 else kt_t1
                        pv_t = pvp.tile([65, NI], F32, tag="pv")
                        for bi, jc0 in enumerate(range(0, NJC, JBATCH)):
                            pts = []
                            st_list = []
                            for jc in range(jc0, jc0 + JBATCH):
                                st_ps = ps.tile([128, NI], F32, tag="big")
                                st_list.append(st_ps)
                                for ih in range(2):
                                    nc.tensor.matmul(
                                        st_ps[:, ih * 512:(ih + 1) * 512],
                                        kt[:, jc],
                                        qt[:, ih * 512:(ih + 1) * 512],
                                        start=True, stop=True)
                            for k, jc in enumerate(range(jc0, jc0 + JBATCH)):
                                pt = ptp.tile([128, NI], BF16, tag="pt")
                                pts.append(pt)
                                nc.scalar.activation(pt[:], st_list[k][:], EXP,
                                                     scale=1.0)
                            if bi == 0 and h + 1 < H:
                                emit_q(h + 1)
                            if bi == 1 and h + 1 < H:
                                emit_k(h + 1)
                            for k, jc in enumerate(range(jc0, jc0 + JBATCH)):
                                for ih in range(2):
                                    nc.tensor.matmul(
                                        pv_t[:, ih * 512:(ih + 1) * 512],
                                        vv[:, jc, h * 65:(h + 1) * 65],
                                        pts[k][:, ih * 512:(ih + 1) * 512],
                                        start=(jc == 0), stop=(jc == NJC - 1))
                        den = np_.tile([1, NI], F32, tag="den")
                        nc.vector.tensor_copy(den[:], pv_t[64:65, :])
                        rrow = np_.tile([1, NI], F32, tag="rrow")
                        nc.vector.reciprocal_approx_fast(rrow[:], den[:])
                        rb = np_.tile([64, NI], F32, tag="rb")
                        nc.gpsimd.partition_broadcast(rb[:], rrow[:])
                        nc.vector.tensor_tensor(
                            out=outt[hr:hr + 64, hp, :],
                            in0=pv_t[0:64, :], in1=rb[:],
                            op=mybir.AluOpType.mult)

                # ---------- stage 3: output projection ----------
                for ib in range(NI // 128):
                    y_ps = ps.tile([128, MODEL], F32, tag="big")
                    for ch in range(NCH):
                        nc.tensor.matmul(y_ps[:],
                                         outt[:, ch, ib * 128:(ib + 1) * 128],
                                         wo[:, ch],
                                         start=(ch == 0), stop=(ch == NCH - 1))
                    y_sb = yp_sb.tile([128, MODEL], F32, tag="ysb")
                    nc.vector.tensor_tensor(out=y_sb[:], in0=y_ps[:], in1=bo_b[:],
                                            op=mybir.AluOpType.add)
                    dma(ib, y_out[ib * 128:(ib + 1) * 128, :], y_sb[:])

    nc.compile()
    return nc


def _get_compiled():
    global _COMPILED
    if _COMPILED is None:
        _COMPILED = _build()
    return _COMPILED


def kernel(x, Wq, Wk, Wv, Wo, bo, rel_content_bias, _trace=False):
    from concourse.bass_utils import run_bass_kernel_spmd
    import ml_dtypes

    nc = _get_compiled()

    x = np.asarray(x, dtype=np.float32)
    Wq = np.asarray(Wq, dtype=np.float32)
    Wk = np.asarray(Wk, dtype=np.float32)
    Wv = np.asarray(Wv, dtype=np.float32)
    Wo = np.asarray(Wo, dtype=np.float32)
    bo = np.asarray(bo, dtype=np.float32)
    bias = np.asarray(rel_content_bias, dtype=np.float32).reshape(H, DK)

    Wq_s = (Wq * SCALE).astype(np.float32)
    def rep2(w):  # [512, h*64] -> [512, h*128] with each head's 64 cols doubled
        w3 = w.reshape(MODEL, H, DK)
        return np.concatenate([w3, w3], axis=2).reshape(MODEL, H * 128)
    Wq_s = rep2(Wq_s)
    relb = np.concatenate([bias.T, bias.T], axis=0).astype(np.float32)  # [128, H]
    onesb = np.ones((128, NJC * H), ml_dtypes.bfloat16)
    Wk_r = rep2(Wk)
    shared = {"wq": Wq_s, "wk": Wk_r, "wv": Wv, "relb": relb, "wo": Wo,
              "bo": bo[None, :], "onesb": onesb}

    in_maps = []
    for c in range(8):
        b, half = c // 2, c % 2
        xt = np.ascontiguousarray(x[b].T)              # [512, 2048]
        if half:
            xt = np.ascontiguousarray(np.roll(xt, -NI, axis=1))
        in_maps.append({"xt": xt, **shared})

    res = run_bass_kernel_spmd(nc, in_maps, core_ids=list(range(8)),
                               trace=_trace)
    out = np.empty((B, N, MODEL), np.float32)
    for c in range(8):
        b, half = c // 2, c % 2
        out[b, half * NI:(half + 1) * NI, :] = res.results[c]["y"]
    if _trace:
        return out, res
    return out


# revision 33
# speedup vs baseline: 1.0099x; 1.0099x over previous
"""Trainium2 Bass kernel for multi-head attention (b=4, n=2048, d=512, h=8, dk=dv=64).

Sharding: 8 cores = 4 batches x 2 query-halves. Each core computes K/V for its
full batch sequence (2048) and attention outputs for its 1024 query rows.
No collectives needed; host stacks the per-core [1024, 512] outputs.

Per-core dataflow (f32r = TF32-like fast fp32 matmul mode; PV in bf16):
  x^T [512, 2048] staged in SBUF.
  Q^T per head, replicated to both partition halves: qt2 [128, h, i]
  K^T per head stored block-diagonally per j-chunk: kt_bd [128, h, jc, 128]
    rows 0:64  = K^T dims x even 64-j half   (cols 0:64)
    rows 64:128= K^T dims x odd 64-j half    (cols 64:128), zeros elsewhere
  -> S^T matmul has K=128 (full PE rate): out rows = 128 consecutive j's.
  V   = x Wv  (+ ones col) [per j-chunk: 128j, 8h*65] in bf16
  c   = x (Wk_h @ rel_bias_h) [per j-chunk: 128j, 8h]  (bias term of logits)
  P^T = exp(S^T + c[j])   (no max-subtraction: logits < ~50)
  outT[65, i] accumulated over j-chunks via lhsT=V_aug, rhs=P^T; row 64 = denom
  out^T scaled by 1/denom (reciprocal_approx_fast + partition broadcast)
  y = out^T.T @ Wo + bo  (bias via broadcast bo + DVE add on PSUM copy).
"""
import numpy as np

B, N, MODEL = 4, 2048, 512
H, DK = 8, 64
SCALE = DK ** -0.5
NI = 1024          # query rows per core
NCH = MODEL // 128  # model-dim chunks
NJC = N // 128      # key/value chunks
NHP = H // 2        # head pairs
JBATCH = 4          # j-chunks per ST/PV batch

_COMPILED = None


def _build():
    import concourse.bass as bass
    from concourse import bacc
    import concourse.mybir as mybir
    import concourse.tile as tile

    F32 = mybir.dt.float32
    F32R = mybir.dt.float32r
    BF16 = mybir.dt.bfloat16
    EXP = mybir.ActivationFunctionType.Exp

    nc = bacc.Bacc("TRN2", target_bir_lowering=False, debug=False, num_devices=8)
    xt_in = nc.dram_tensor("xt", [MODEL, N], F32R, kind="ExternalInput")
    wq_in = nc.dram_tensor("wq", [MODEL, H * 128], F32R, kind="ExternalInput")
    wk_in = nc.dram_tensor("wk", [MODEL, H * 128], F32R, kind="ExternalInput")
    wv_in = nc.dram_tensor("wv", [MODEL, MODEL], F32R, kind="ExternalInput")
    relb_in = nc.dram_tensor("relb", [128, H], F32, kind="ExternalInput")
    wo_in = nc.dram_tensor("wo", [MODEL, MODEL], F32R, kind="ExternalInput")
    bo_in = nc.dram_tensor("bo", [1, MODEL], F32, kind="ExternalInput")
    onesb_in = nc.dram_tensor("onesb", [128, NJC * H], BF16, kind="ExternalInput")
    y_out = nc.dram_tensor("y", [NI, MODEL], F32, kind="ExternalOutput")

    HALF = N // 2

    with tile.TileContext(nc) as tc:
        with (
            tc.tile_pool(name="w", bufs=1) as wp,
            tc.tile_pool(name="acts", bufs=1) as ap,
            tc.tile_pool(name="big", bufs=2, space="PSUM") as ps,
            tc.tile_pool(name="qk", bufs=1, space="PSUM") as qkp,
            tc.tile_pool(name="pv", bufs=1, space="PSUM") as pvp,
        ):
            # ---------- persistent tiles ----------
            wo = wp.tile([128, NCH, MODEL], F32R, tag="wo")
            bo = wp.tile([1, MODEL], F32, tag="bo")
            bo_b = wp.tile([128, MODEL], F32, tag="bo_b")
            vv = ap.tile([128, NJC, H * 65], BF16, tag="vv")
            relb = ap.tile([128, H], F32, tag="relb")
            outt = ap.tile([128, NHP, NI], F32R, tag="outt")
            xt0 = ap.tile([128, NCH, 512], F32R, tag="xt0")
            xt1 = ap.tile([128, NCH, 512], F32R, tag="xt1")
            xt2 = ap.tile([128, NCH, 512], F32R, tag="xt2")
            xt3 = ap.tile([128, NCH, 512], F32R, tag="xt3")
            xts = [xt0, xt1, xt2, xt3]
            wq = ap.tile([128, NCH, H * 128], F32R, tag="wq")
            wk = ap.tile([128, NCH, H * 128], F32R, tag="wk")

            def r3(d):
                return d[:].rearrange("(c p) n -> p c n", p=128)

            def dma(i, out, in_, engs=None):
                engs = engs or (nc.sync, nc.gpsimd)
                engs[i % len(engs)].dma_start(out=out, in_=in_)

            xsrc = r3(xt_in)
            for q in range(4):
                for chh in range(2):
                    dma(q * 2 + chh,
                        xts[q][:, chh * 2:(chh + 1) * 2, :],
                        xsrc[:, chh * 2:(chh + 1) * 2, q * 512:(q + 1) * 512],
                        engs=(nc.sync, nc.gpsimd, nc.scalar))
            nc.sync.dma_start(out=bo[:], in_=bo_in[:])
            nc.gpsimd.partition_broadcast(bo_b[:], bo[:])
            # ones columns of V_aug: contiguous DMA to scratch, strided DVE copy
            onesb_t = wp.tile([128, NJC * H], BF16, tag="onesb")
            nc.sync.dma_start(out=onesb_t[:], in_=onesb_in[:])
            nc.vector.tensor_copy(
                vv[:].rearrange("p j (h e) -> p (j h) e", e=65)[:, :, 64:65],
                onesb_t[:].rearrange("p (n o) -> p n o", o=1))
            def xtv(ch, start, size):
                t = xts[start // 512]
                off = start % 512
                assert off + size <= 512
                return t[:, ch, off:off + size]


            kt_t0 = ap.tile([128, NJC, 128], F32R, tag="kt0")
            kt_t1 = ap.tile([128, NJC, 128], F32R, tag="kt1")
            qt_t0 = ap.tile([128, NI], F32R, tag="qt0")
            qt_t1 = ap.tile([128, NI], F32R, tag="qt1")
            with (
                tc.tile_pool(name="pt", bufs=10) as ptp,
                tc.tile_pool(name="norm", bufs=2) as np_,
                tc.tile_pool(name="ysb", bufs=2) as yp_sb,
            ):
                # ---- V and c projections (vv also carries the exp bias c) ----
                with tc.tile_pool(name="s1v", bufs=1) as s1v:
                    wv = s1v.tile([128, NCH, MODEL], F32R, tag="wv")

                    zeros = s1v.tile([128, 512], F32, tag="zeros")
                    nc.vector.memset(zeros[:], 0.0)
                    for ktz in (kt_t0, kt_t1):
                        for jh in range(2):
                            js = slice(jh * 8, jh * 8 + 8)
                            nc.vector.tensor_copy(
                                ktz[0:64, js, 64:128],
                                zeros[0:64].rearrange("p (j m) -> p j m", m=64))
                            nc.vector.tensor_copy(
                                ktz[64:128, js, 0:64],
                                zeros[64:128].rearrange("p (j m) -> p j m", m=64))
                    for ch in range(NCH):
                        dma(ch, wv[:, ch], r3(wv_in)[:, ch])
                    nc.sync.dma_start(out=relb[:], in_=relb_in[:])
                    for ch in range(NCH):
                        dma(ch, wq[:, ch], r3(wq_in)[:, ch])
                        dma(ch + 1, wk[:, ch], r3(wk_in)[:, ch])
                        dma(ch, wo[:, ch], r3(wo_in)[:, ch])
                    for jc in range(NJC):
                        v_ps = ps.tile([128, NI], F32, tag="big")
                        for ch in range(NCH):
                            nc.tensor.matmul(v_ps[:, 0:MODEL],
                                             xtv(ch, jc * 128, 128),
                                             wv[:, ch],
                                             start=(ch == 0), stop=(ch == NCH - 1))
                        nc.vector.tensor_copy(
                            vv[:, jc].rearrange("p (h e) -> p h e", e=65)[:, :, 0:64],
                            v_ps[:, 0:MODEL].rearrange("p (h e) -> p h e", e=64))

                    # ---- merged per-head: Q^T, K^T(block-diag), ST/exp/PV ----
                    def emit_qk(h):
                        qt = qt_t0 if h % 2 == 0 else qt_t1
                        kt = kt_t0 if h % 2 == 0 else kt_t1
                        q_ps = qkp.tile([128, NI], F32, tag="qk")
                        for ib in range(NI // 512):
                            for ch in range(NCH):
                                nc.tensor.matmul(
                                    q_ps[:, ib * 512:(ib + 1) * 512],
                                    wq[:, ch, h * 128:(h + 1) * 128],
                                    xtv(ch, ib * 512, 512),
                                    start=(ch == 0), stop=(ch == NCH - 1))
                        nc.vector.tensor_scalar_add(qt[:], q_ps[:], relb[:, h:h + 1])
                        for jb in range(N // NI):
                            k_ps = qkp.tile([128, NI], F32, tag="qk")
                            for sb in range(NI // 512):
                                off = jb * NI + sb * 512
                                for ch in range(NCH):
                                    nc.tensor.matmul(
                                        k_ps[:, sb * 512:(sb + 1) * 512],
                                        wk[:, ch, h * 128:(h + 1) * 128],
                                        xtv(ch, off, 512),
                                        start=(ch == 0), stop=(ch == NCH - 1))
                            kp = k_ps[:].rearrange("p (t e c) -> p t e c", t=8, e=2)
                            jcs = slice(jb * 8, jb * 8 + 8)
                            nc.vector.tensor_copy(kt[0:64, jcs, 0:64],
                                                  kp[0:64, :, 0])
                            nc.vector.tensor_copy(kt[64:128, jcs, 64:128],
                                                  kp[64:128, :, 1])

                    emit_qk(0)
                    for h in range(H):
                        hp, hr = h // 2, (h % 2) * 64
                        qt = qt_t0 if h % 2 == 0 else qt_t1
                        kt = kt_t0 if h % 2 == 0 else kt_t1
                        pv_t = pvp.tile([65, NI], F32, tag="pv")
                        for bi, jc0 in enumerate(range(0, NJC, JBATCH)):
                            pts = []
                            st_list = []
                            for jc in range(jc0, jc0 + JBATCH):
                                st_ps = ps.tile([128, NI], F32, tag="big")
                                st_list.append(st_ps)
                                for ih in range(2):
                                    nc.tensor.matmul(
                                        st_ps[:, ih * 512:(ih + 1) * 512],
                                        kt[:, jc],
                                        qt[:, ih * 512:(ih + 1) * 512],
                                        start=True, stop=True)
                            for k, jc in enumerate(range(jc0, jc0 + JBATCH)):
                                pt = ptp.tile([128, NI], BF16, tag="pt")
                                pts.append(pt)
                                nc.scalar.activation(pt[:], st_list[k][:], EXP,
                                                     scale=1.0)
                            if bi == 0 and h + 1 < H:
                                emit_qk(h + 1)
                            for k, jc in enumerate(range(jc0, jc0 + JBATCH)):
                                for ih in range(2):
                                    nc.tensor.matmul(
                                        pv_t[:, ih * 512:(ih + 1) * 512],
                                        vv[:, jc, h * 65:(h + 1) * 65],
                                        pts[k][:, ih * 512:(ih + 1) * 512],
                                        start=(jc == 0), stop=(jc == NJC - 1))
                        den = np_.tile([1, NI], F32, tag="den")
                        nc.vector.tensor_copy(den[:], pv_t[64:65, :])
                        rrow = np_.tile([1, NI], F32, tag="rrow")
                        nc.vector.reciprocal_approx_fast(rrow[:], den[:])
                        rb = np_.tile([64, NI], F32, tag="rb")
                        nc.gpsimd.partition_broadcast(rb[:], rrow[:])
                        nc.vector.tensor_tensor(
                            out=outt[hr:hr + 64, hp, :],
                            in0=pv_t[0:64, :], in1=rb[:],
                            op=mybir.AluOpType.mult)

                # ---------- stage 3: output projection ----------
                for ib in range(NI // 128):
                    y_ps = ps.tile([128, MODEL], F32, tag="big")
                    for ch in range(NCH):
                        nc.tensor.matmul(y_ps[:],
                                         outt[:, ch, ib * 128:(ib + 1) * 128],
                                         wo[:, ch],
                                         start=(ch == 0), stop=(ch == NCH - 1))
                    y_sb = yp_sb.tile([128, MODEL], F32, tag="ysb")
                    nc.vector.tensor_tensor(out=y_sb[:], in0=y_ps[:], in1=bo_b[:],
                                            op=mybir.AluOpType.add)
                    dma(ib, y_out[ib * 128:(ib + 1) * 128, :], y_sb[:])

    nc.compile()
    return nc


def _get_compiled():
    global _COMPILED
    if _COMPILED is None:
        _COMPILED = _build()
    return _COMPILED


def kernel(x, Wq, Wk, Wv, Wo, bo, rel_content_bias, _trace=False):
    from concourse.bass_utils import run_bass_kernel_spmd
    import ml_dtypes

    nc = _get_compiled()

    x = np.asarray(x, dtype=np.float32)
    Wq = np.asarray(Wq, dtype=np.float32)
    Wk = np.asarray(Wk, dtype=np.float32)
    Wv = np.asarray(Wv, dtype=np.float32)
    Wo = np.asarray(Wo, dtype=np.float32)
    bo = np.asarray(bo, dtype=np.float32)
    bias = np.asarray(rel_content_bias, dtype=np.float32).reshape(H, DK)

    Wq_s = (Wq * SCALE).astype(np.float32)
    def rep2(w):  # [512, h*64] -> [512, h*128] with each head's 64 cols doubled
        w3 = w.reshape(MODEL, H, DK)
        return np.concatenate([w3, w3], axis=2).reshape(MODEL, H * 128)
    Wq_s = rep2(Wq_s)
    relb = np.concatenate([bias.T, bias.T], axis=0).astype(np.float32)  # [128, H]
    onesb = np.ones((128, NJC * H), ml_dtypes.bfloat16)
    Wk_r = rep2(Wk)
    shared = {"wq": Wq_s, "wk": Wk_r, "wv": Wv, "relb": relb, "wo": Wo,
              "bo": bo[None, :], "onesb": onesb}

    in_maps = []
    for c in range(8):
        b, half = c // 2, c % 2
        xt = np.ascontiguousarray(x[b].T)              # [512, 2048]
        if half:
            xt = np.ascontiguousarray(np.roll(xt, -NI, axis=1))
        in_maps.append({"xt": xt, **shared})

    res = run_bass_kernel_spmd(nc, in_maps, core_ids=list(range(8)),
                               trace=_trace)
    out = np.empty((B, N, MODEL), np.float32)
    for c in range(8):
        b, half = c // 2, c % 2
        out[b, half * NI:(half + 1) * NI, :] = res.results[c]["y"]
    if _trace:
        return out, res
    return out
